# revision 14
# baseline (speedup 1.0000x reference)
"""Causal self-attention on 8 TRN2 NeuronCores.

Sharding: core c handles batch b = c//2 and head-group g = c%2 (8 of 16 heads).
Each core computes its partial y^T = w_proj[slice].T @ o^T (contraction over its
512 o-channels); the host sums the two partials per batch and adds b_proj.

Shapes (hardcoded): B=4, T=2048, C=1024, H=16, HD=64.
"""

import numpy as np

B, T, C, H = 4, 2048, 1024, 16
HD = C // H          # 64
G = 2                # head groups
NHL = H // G         # 8 heads per core
GQ = NHL * HD        # 512 channel slice per core
P = 128
NT = T // P          # 16 token tiles / k-chunks
NCHUNK = C // P      # 8 contraction chunks for qkv
SCALE = 1.0 / float(np.sqrt(HD))

_PROGRAM = None


def _emit(ctx, tc, aps, mybir, bass):
    import contextlib

    nc = tc.nc
    f32 = mybir.dt.float32
    f32r = mybir.dt.float32r
    bf16 = mybir.dt.bfloat16
    EXP = mybir.ActivationFunctionType.Exp

    x_d, wqkv_d, bqk_d, bv_d, wp_d, yT_d = (
        aps["x"], aps["wqkv"], aps["bqk"], aps["bv"], aps["wp"], aps["yT"],
    )

    # ---------------- pools ----------------
    const = ctx.enter_context(tc.tile_pool(name="const", bufs=1))
    dramp = ctx.enter_context(tc.tile_pool(name="dramp", bufs=1, space="DRAM"))
    # psum: 2 + 4 + 2 = 8 banks
    ps_ab = ctx.enter_context(tc.tile_pool(name="ps_ab", bufs=2, space="PSUM"))
    ps_sc = ctx.enter_context(tc.tile_pool(name="ps_sc", bufs=2, space="PSUM"))
    ps_pv = ctx.enter_context(tc.tile_pool(name="ps_pv", bufs=2, space="PSUM"))

    qkp = ctx.enter_context(tc.tile_pool(name="qkp", bufs=8))
    vap = ctx.enter_context(tc.tile_pool(name="vap", bufs=16))
    ptp = ctx.enter_context(tc.tile_pool(name="ptp", bufs=2))
    otp = ctx.enter_context(tc.tile_pool(name="otp", bufs=2))
    rcp = ctx.enter_context(tc.tile_pool(name="rcp", bufs=1))

    # constants
    identity = const.tile([P, P], f32)
    from concourse.masks import make_identity
    make_identity(nc, identity)
    bqk_sb = const.tile([P, 8], f32)
    nc.sync.dma_start(bqk_sb[:], bqk_d[:])
    bvb = const.tile([P, GQ], f32)
    nc.sync.dma_start(bvb[:], bv_d[None, :].to_broadcast((P, GQ)))
    ones8 = const.tile([P, NHL, 1], f32)
    nc.vector.memset(ones8[:], 1.0)

    odram = dramp.tile([GQ, T], f32r, space="DRAM")

    # ---------------- phase A: load x, build xT ----------------
    stackAB = contextlib.ExitStack()
    xTp = stackAB.enter_context(tc.tile_pool(name="xTp", bufs=8))
    wqkp = stackAB.enter_context(tc.tile_pool(name="wqkp", bufs=8))
    wvp = stackAB.enter_context(tc.tile_pool(name="wvp", bufs=1))
    stackA = contextlib.ExitStack()
    xp = stackA.enter_context(tc.tile_pool(name="xp", bufs=2))

    xT = []  # 8 tiles [128 c, 2048 t] f32
    for r in range(NCHUNK):
        t_ = xTp.tile([P, T], f32r, name=f"xT{r}", tag="xT")
        xT.append(t_)

    # x tile t -> for each r, transpose block into psum, 2 t-blocks per psum tile
    for tg in range(NT // 2):  # groups of 2 t-tiles
        xts = []
        for tt in range(2):
            t = 2 * tg + tt
            x_t = xp.tile([P, C], f32, name=f"x_{t}", tag="x")
            nc.sync.dma_start(x_t[:], x_d[t * P:(t + 1) * P, :])
            xts.append(x_t)
        for r in range(NCHUNK):
            tp = ps_ab.tile([P, 256], f32, name=f"tp_{tg}_{r}", tag="ps_ab")
            for tt in range(2):
                nc.tensor.transpose(
                    tp[:, tt * P:(tt + 1) * P],
                    xts[tt][:, r * P:(r + 1) * P],
                    identity,
                )
            nc.vector.tensor_copy(xT[r][:, tg * 256:(tg + 1) * 256], tp[:])
    stackA.close()

    # ---------------- phase B: qkv ----------------
    # q/k transposed: for ct in 0..7 (4 q-tiles then 4 k-tiles),
    # out tile [128 c', 2048 t] accumulating 8 chunks, 4 t-windows of 512.
    qkT = []  # bf16 tiles; 0..3 = qT, 4..7 = kT
    for ct in range(8):
        o_t = qkp.tile([P, T], bf16, name=f"qkT{ct}", tag="qkT")
        qkT.append(o_t)

    # v natural + ones col: vaug[t-tile] = [128 t, 8 heads, 65]
    vaug = []
    for t in range(NT):
        va = vap.tile([P, NHL, HD + 1], f32r, name=f"vaug{t}", tag="vaug")
        nc.vector.tensor_copy(va[:, :, HD:HD + 1], ones8[:])
        vaug.append(va)

    # emission order: q0,k0 first so attention can start early, then v, then rest
    wqkv_r = wqkv_d.rearrange("(a p) n -> p a n", p=P)  # [128, 8, 1536]

    def emit_qk_tile(ct):
        # ct in 0..7 -> col range in wqkv slice: q tiles 0..3 -> cols 128*ct,
        # k tiles -> 512 + 128*(ct-4); source layout is [q(512) k(512) v(512)]
        col0 = ct * P
        w_t = wqkp.tile([P, NCHUNK, P], f32r, name=f"wqk_{ct}", tag="wqk")
        nc.sync.dma_start(w_t[:], wqkv_r[:, :, col0:col0 + P])
        for tw in range(4):
            ps = ps_ab.tile([P, 512], f32, name=f"qkps_{ct}_{tw}", tag="ps_ab")
            for a in range(NCHUNK):
                nc.tensor.matmul(
                    ps[:],
                    w_t[:, a, :],
                    xT[a][:, tw * 512:(tw + 1) * 512],
                    start=(a == 0),
                    stop=(a == NCHUNK - 1),
                )
            # add bias (per-partition) and cast to bf16
            nc.vector.tensor_scalar_add(
                qkT[ct][:, tw * 512:(tw + 1) * 512], ps[:], bqk_sb[:, ct:ct + 1]
            )

    def emit_v():
        w_t = wvp.tile([P, NCHUNK, GQ], f32r, name="wv", tag="wv")
        nc.sync.dma_start(w_t[:], wqkv_r[:, :, 2 * GQ:3 * GQ])
        for t in range(NT):
            ps = ps_ab.tile([P, GQ], f32, name=f"vps_{t}", tag="ps_ab")
            for a in range(NCHUNK):
                nc.tensor.matmul(
                    ps[:],
                    xT[a][:, t * P:(t + 1) * P],
                    w_t[:, a, :],
                    start=(a == 0),
                    stop=(a == NCHUNK - 1),
                )
            nc.vector.tensor_add(
                vaug[t][:, :, 0:HD],
                ps[:].rearrange("p (h d) -> p h d", h=NHL),
                bvb[:].rearrange("p (h d) -> p h d", h=NHL),
            )

    emit_qk_tile(0)  # q tile 0
    emit_qk_tile(4)  # k tile 0
    emit_v()
    for j in range(1, 4):
        emit_qk_tile(j)
        emit_qk_tile(4 + j)

    # ---------------- phase C: attention ----------------
    # per head h: q rows = qkT[h//2][(h%2)*64 : +64], k rows same in qkT[4+h//2]
    for h in range(NHL):
        qt = qkT[h // 2]
        kt = qkT[4 + h // 2]
        r0 = (h % 2) * HD
        ot_tiles = []
        for half in range(2):
            qlo = half * 1024
            qhi = qlo + 1024
            nch = qhi // P  # chunks in this half
            # PV psum windows (2 per half)
            pv = []
            for w in range(2):
                pvt = ps_pv.tile([P, 512], f32, name=f"pv_{h}_{half}_{w}", tag="ps_pv")
                pv.append(pvt)
            for i in range(nch):
                qs = max(qlo, i * P)
                fd = qhi - qs
                sc = ps_sc.tile([P, 1024], f32, name=f"sc_{h}_{half}_{i}", tag="ps_sc")
                # scores^T [k, q] pieces of <=512 within psum banks
                off0 = qs - qlo
                for pw in range(2):
                    ws = qlo + pw * 512
                    we = ws + 512
                    s = max(qs, ws)
                    if s >= we:
                        continue
                    nc.tensor.matmul(
                        sc[:, s - qlo:we - qlo],
                        kt[r0:r0 + HD, i * P:(i + 1) * P],
                        qt[r0:r0 + HD, s:we],
                        start=True,
                        stop=True,
                    )
                pt = ptp.tile([P, 1024], f32r, name=f"pt_{h}_{half}_{i}", tag="pt")
                nc.scalar.activation(pt[:, off0:], sc[:, off0:], EXP, scale=SCALE)
                if i * P >= qlo:
                    # diagonal block: zero entries where q < k
                    nc.gpsimd.affine_select(
                        out=pt[:, i * P - qlo:i * P - qlo + P],
                        in_=pt[:, i * P - qlo:i * P - qlo + P],
                        compare_op=mybir.AluOpType.is_ge,
                        fill=0.0,
                        base=0,
                        pattern=[[1, P]],
                        channel_multiplier=-1,
                    )
                # PV accumulation into windows
                for w in range(2):
                    ws = qlo + w * 512
                    we = ws + 512
                    s = max(qs, ws)
                    if s >= we:
                        continue
                    nc.tensor.matmul(
                        pv[w][0:HD + 1, s - ws:],
                        vaug[i][:, h, :],
                        pt[:, s - qlo:we - qlo],
                        start=(i == 0),
                        stop=(i == we // P - 1),
                    )
            # normalize: recip of denominator row, broadcast, multiply
            ot = otp.tile([HD, 1024], f32r, name=f"ot_{h}_{half}", tag="ot")
            ot_tiles.append(ot)
            for w in range(2):
                rc = rcp.tile([1, 512], f32, name=f"rc_{h}_{half}_{w}", tag="rc")
                nc.vector.reciprocal(rc[:], pv[w][HD:HD + 1, :])
                rcb = rcp.tile([HD, 512], f32, name=f"rcb_{h}_{half}_{w}", tag="rcb")
                nc.gpsimd.partition_broadcast(rcb[:], rc[:])
                nc.vector.tensor_mul(
                    ot[:, w * 512:(w + 1) * 512], pv[w][0:HD, :], rcb[:]
                )
            nc.sync.dma_start(odram[h * HD:(h + 1) * HD, qlo:qhi], ot[:])

    stackAB.close()  # release x / w pools (xT stays for LIFO ordering)

    # ---------------- phase D: proj ----------------
    stackD = contextlib.ExitStack()
    orp = stackD.enter_context(tc.tile_pool(name="orp", bufs=4))
    wpp = stackD.enter_context(tc.tile_pool(name="wpp", bufs=1))
    ysp = stackD.enter_context(tc.tile_pool(name="ysp", bufs=3))

    wp_t = wpp.tile([P, 4, C], f32r, name="wp", tag="wp")
    nc.sync.dma_start(wp_t[:], wp_d.rearrange("(a p) n -> p a n", p=P))
    oTr = []
    for a in range(4):
        o_t = orp.tile([P, T], f32r, name=f"oTr{a}", tag="oTr")
        nc.sync.dma_start(o_t[:], odram[a * P:(a + 1) * P, :])
        oTr.append(o_t)
    for m in range(NCHUNK):  # cout tiles
        for tw in range(4):
            ps = ps_ab.tile([P, 512], f32, name=f"yps_{m}_{tw}", tag="ps_ab")
            for a in range(4):
                nc.tensor.matmul(
                    ps[:],
                    wp_t[:, a, m * P:(m + 1) * P],
                    oTr[a][:, tw * 512:(tw + 1) * 512],
                    start=(a == 0),
                    stop=(a == 3),
                )
            ys = ysp.tile([P, 512], f32, name=f"ys_{m}_{tw}", tag="ys")
            nc.vector.tensor_copy(ys[:], ps[:])
            nc.sync.dma_start(
                yT_d[m * P:(m + 1) * P, tw * 512:(tw + 1) * 512], ys[:]
            )
    stackD.close()


def _build_program():
    import contextlib

    import concourse.bass as bass
    import concourse.mybir as mybir
    import concourse.tile as tile
    from concourse import bacc

    nc = bacc.Bacc("TRN2", target_bir_lowering=False, debug=False, num_devices=8)
    f32 = mybir.dt.float32
    aps = {
        "x": nc.dram_tensor("x", [T, C], f32, kind="ExternalInput").ap(),
        "wqkv": nc.dram_tensor("wqkv", [C, 3 * GQ], mybir.dt.float32r, kind="ExternalInput").ap(),
        "bqk": nc.dram_tensor("bqk", [P, 8], f32, kind="ExternalInput").ap(),
        "bv": nc.dram_tensor("bv", [GQ], f32, kind="ExternalInput").ap(),
        "wp": nc.dram_tensor("wp", [GQ, C], mybir.dt.float32r, kind="ExternalInput").ap(),
        "yT": nc.dram_tensor("yT", [C, T], f32, kind="ExternalOutput").ap(),
    }
    with tile.TileContext(nc) as tc:
        with contextlib.ExitStack() as ctx:
            _emit(ctx, tc, aps, mybir, bass)
    nc.compile()
    return nc


def get_program():
    global _PROGRAM
    if _PROGRAM is None:
        _PROGRAM = _build_program()
    return _PROGRAM


def make_in_maps(x, w_qkv, b_qkv, w_proj):
    x = np.asarray(x, np.float32)
    w_qkv = np.asarray(w_qkv, np.float32)
    b_qkv = np.asarray(b_qkv, np.float32)
    w_proj = np.asarray(w_proj, np.float32)
    in_maps = []
    for c in range(8):
        b = c // 2
        g = c % 2
        q0 = g * GQ
        wq = w_qkv[:, q0:q0 + GQ]
        wk = w_qkv[:, C + q0:C + q0 + GQ]
        wv = w_qkv[:, 2 * C + q0:2 * C + q0 + GQ]
        wqkv = np.ascontiguousarray(np.concatenate([wq, wk, wv], axis=1))
        bq = b_qkv[q0:q0 + GQ]
        bk = b_qkv[C + q0:C + q0 + GQ]
        bqk = np.ascontiguousarray(np.concatenate([bq, bk]).reshape(8, P).T)
        bv = np.ascontiguousarray(b_qkv[2 * C + q0:2 * C + q0 + GQ])
        in_maps.append({
            "x": np.ascontiguousarray(x[b]),
            "wqkv": wqkv,
            "bqk": bqk,
            "bv": bv,
            "wp": np.ascontiguousarray(w_proj[q0:q0 + GQ, :]),
        })
    return in_maps


def combine_outputs(outs, b_proj):
    b_proj = np.asarray(b_proj, np.float32)
    y = np.empty((B, T, C), np.float32)
    for b in range(B):
        acc = outs[2 * b] + outs[2 * b + 1]  # [C, T]
        y[b] = acc.T + b_proj
    return y


def kernel(x, w_qkv, b_qkv, w_proj, b_proj, _trace=False):
    from concourse import bass_utils

    nc = get_program()
    in_maps = make_in_maps(x, w_qkv, b_qkv, w_proj)
    res = bass_utils.run_bass_kernel_spmd(
        nc, in_maps, core_ids=list(range(8)), trace=_trace
    )
    outs = [r["yT"] for r in res.results]
    y = combine_outputs(outs, b_proj)
    if _trace:
        return y, res
    return y
